# revision 1
# baseline (speedup 1.0000x reference)
"""Causal multi-head self-attention on 8 Trainium2 NeuronCores.

Sharding: tensor-parallel over heads. Each of the 8 cores owns 2 heads
(128 of the 1024 qkv dims). Per core:
  - QT/KT = (x @ Wq_c^T)^T etc. computed in transposed-activation layout
    [128 dims, 8192 tokens] (Wq pre-scaled by 1/sqrt(hd) on host).
  - V transposed back to natural [tokens, dims] via PE transpose, with a
    ones-column appended so the attn@V matmul also produces the softmax
    denominator (softmax computed without max-subtraction: scores are
    O(+-10) so exp() is safe in fp32).
  - scores^T = K Q^T per (batch, head), causal via per-tile widths + one
    128x128 staircase mask on the diagonal tiles.
  - out_partial = A_c^T @ Wo_c^T written per core; host sums the 8
    partials (the row-parallel all-reduce done on host).

All matmuls run as float32r (full-rate PE mode, fp32 storage); set
MM_DT = "f32" for exact-fp32 (4x slower) matmuls.
"""

import numpy as np
from contextlib import ExitStack

import concourse.bass as bass
import concourse.mybir as mybir
import concourse.tile as tile
from concourse import bacc

F32 = mybir.dt.float32
F32R = mybir.dt.float32r
BF16 = mybir.dt.bfloat16
EXP = mybir.ActivationFunctionType.Exp
MULT = mybir.AluOpType.mult


class Cfg:
    def __init__(self, B=4, S=2048, D=1024, TCH=512, QCH=512, mm_dt="f32r"):
        self.B, self.S, self.D = B, S, D
        self.T = B * S
        self.KT = D // 128          # contraction tiles for projections
        self.TCH = TCH              # token chunk for projections
        self.QCH = QCH              # query chunk for attention
        self.NQC = S // QCH         # q chunks per batch
        self.HD = 64
        self.mm_dt = mm_dt
        assert S % QCH == 0 and QCH % 128 == 0 and self.T % TCH == 0
        assert D % 512 == 0 or D == 256


def _mmdt(cfg):
    return {"f32r": F32R, "bf16": BF16, "f32": F32}[cfg.mm_dt]


def build_program(cfg: Cfg, dbg=False):
    """Build the SPMD single-core Bass program (same program all cores)."""
    nc = bacc.Bacc("TRN2", target_bir_lowering=False, debug=False)
    B, S, D, T, KT = cfg.B, cfg.S, cfg.D, cfg.T, cfg.KT
    TCH, QCH, NQC = cfg.TCH, cfg.QCH, cfg.NQC
    NVT = T // 128                 # number of 128-token V tiles
    MMDT = _mmdt(cfg)              # fp32r storage: matmul consumers require
                                   # producers to declare rounded output
    MMDT_G = MMDT

    xT_d = nc.dram_tensor("xT", [128, KT, T], MMDT, kind="ExternalInput")
    wq_d = nc.dram_tensor("wq", [128, KT, 128], MMDT, kind="ExternalInput")
    wk_d = nc.dram_tensor("wk", [128, KT, 128], MMDT, kind="ExternalInput")
    wv_d = nc.dram_tensor("wv", [128, KT, 128], MMDT, kind="ExternalInput")
    wo_d = nc.dram_tensor("wo", [128, D], MMDT, kind="ExternalInput")
    mask_d = nc.dram_tensor("mask", [128, 128], F32, kind="ExternalInput")
    ident_d = nc.dram_tensor("ident", [128, 128], MMDT_G, kind="ExternalInput")
    out_d = nc.dram_tensor("out_p", [T, D], F32, kind="ExternalOutput")
    out_r = out_d.rearrange("(n p) o -> p n o", p=128)   # [128, NVT, D]
    if dbg:
        dbg_qt = nc.dram_tensor("dbg_qt", [128, T], F32, kind="ExternalOutput")
        dbg_kt = nc.dram_tensor("dbg_kt", [128, T], F32, kind="ExternalOutput")
        dbg_v = nc.dram_tensor("dbg_v", [128, NVT, 130], F32,
                               kind="ExternalOutput")
        dbg_a = nc.dram_tensor("dbg_a", [128, T], F32, kind="ExternalOutput")
        dbg_pt = nc.dram_tensor("dbg_pt", [4, 128, 2 * cfg.QCH], F32,
                                kind="ExternalOutput")
        dbg_att = nc.dram_tensor("dbg_att", [2, 65, cfg.QCH], F32,
                                 kind="ExternalOutput")
        dbg_bc = nc.dram_tensor("dbg_bc", [2, 64, cfg.QCH], F32,
                                kind="ExternalOutput")
        dbg_rc = nc.dram_tensor("dbg_rc", [2, cfg.QCH], F32,
                                kind="ExternalOutput")

    with tile.TileContext(nc) as tc, ExitStack() as ctx:
        persist = ctx.enter_context(tc.tile_pool(name="persist", bufs=1))

        qt_sb = persist.tile([128, T], MMDT, tag="qt")
        kt_sb = persist.tile([128, T], MMDT, tag="kt")
        a_sb = persist.tile([128, T], MMDT, tag="a")
        # V natural layout, one ones-column per head so each attn@V matmul
        # also emits the softmax denominator in its last output row:
        #   cols 0:64 = head0 dims, col 64 = 1.0,
        #   cols 65:129 = head1 dims, col 129 = 1.0
        v_sb = persist.tile([128, NVT, 130], MMDT, tag="v")
        wq_sb = persist.tile([128, KT, 128], MMDT, tag="wq")
        wk_sb = persist.tile([128, KT, 128], MMDT, tag="wk")
        wv_sb = persist.tile([128, KT, 128], MMDT, tag="wv")
        wo_sb = persist.tile([128, D], MMDT, tag="wo")
        mask_sb = persist.tile([128, 128], F32, tag="mask")
        ident = persist.tile([128, 128], MMDT, tag="ident")
        ones128 = persist.tile([128, 1], F32, tag="ones128")
        nc.vector.memset(ones128[:], 1.0)

        nc.sync.dma_start(wq_sb[:], wq_d[:])
        nc.sync.dma_start(wk_sb[:], wk_d[:])
        nc.sync.dma_start(wv_sb[:], wv_d[:])
        nc.sync.dma_start(wo_sb[:], wo_d[:])
        nc.sync.dma_start(mask_sb[:], mask_d[:])
        nc.sync.dma_start(ident[:], ident_d[:])
        nc.vector.tensor_copy(
            v_sb[:, :, 64:65],
            ones128[:, None, :].to_broadcast((128, NVT, 1)))
        nc.vector.tensor_copy(
            v_sb[:, :, 129:130],
            ones128[:, None, :].to_broadcast((128, NVT, 1)))

        # ---------------- Phase 1: Q/K/V projections -------------------
        with tc.tile_pool(name="xp", bufs=2) as xp, \
             tc.tile_pool(name="vtp", bufs=3) as vtp, \
             tc.tile_pool(name="pp", bufs=4, space="PSUM") as pp, \
             tc.tile_pool(name="trp", bufs=2, space="PSUM") as trp:
            for tci in range(T // TCH):
                t0 = tci * TCH
                x_t = xp.tile([128, KT, TCH], MMDT, tag="x")
                nsplit = min(4 if tci == 0 else 2, KT)
                step = KT // nsplit
                for si in range(nsplit):
                    nc.sync.dma_start(
                        x_t[:, si * step:(si + 1) * step, :],
                        xT_d[:, si * step:(si + 1) * step, t0:t0 + TCH])
                for w_sb, kind in ((wq_sb, "q"), (wk_sb, "k"), (wv_sb, "v")):
                    ps = pp.tile([128, TCH], F32, tag="proj")
                    for kt in range(KT):
                        nc.tensor.matmul(
                            ps[:],
                            w_sb[:, kt, :],
                            x_t[:, kt, :],
                            start=(kt == 0), stop=(kt == KT - 1),
                        )
                    if kind == "q":
                        nc.vector.tensor_copy(qt_sb[:, t0:t0 + TCH], ps[:])
                    elif kind == "k":
                        nc.vector.tensor_copy(kt_sb[:, t0:t0 + TCH], ps[:])
                    else:
                        vt_t = vtp.tile([128, TCH], MMDT, tag="vt")
                        nc.vector.tensor_copy(vt_t[:], ps[:])
                        for j in range(TCH // 128):
                            tr = trp.tile([128, 128], MMDT, tag="tr")

                            nc.tensor.transpose(
                                tr[:], vt_t[:, j * 128:(j + 1) * 128], ident[:])
                            ktg = (t0 + j * 128) // 128
                            nc.vector.tensor_copy(
                                v_sb[:, ktg, 0:64], tr[:, 0:64])
                            nc.vector.tensor_copy(
                                v_sb[:, ktg, 65:129], tr[:, 64:128])

        # -------- Phase 2: attention + fused output projection ---------
        OC = min(512, D)
        with tc.tile_pool(name="ptp", bufs=2) as ptp, \
             tc.tile_pool(name="rcp", bufs=2) as rcp, \
             tc.tile_pool(name="op", bufs=2) as op, \
             tc.tile_pool(name="drp", bufs=2, space="DRAM") as drp, \
             tc.tile_pool(name="scp", bufs=2, space="PSUM") as scp, \
             tc.tile_pool(name="attp", bufs=2, space="PSUM") as attp, \
             tc.tile_pool(name="wop", bufs=2, space="PSUM") as wop:
            for b in range(B):
                base = b * S
                vbase = base // 128
                for qc in range(NQC - 1, -1, -1):
                    q0 = qc * QCH
                    n_kt = (q0 + QCH) // 128
                    att0 = attp.tile([65, QCH], F32, tag="att")
                    att1 = attp.tile([65, QCH], F32, tag="att")
                    for kti in range(n_kt):
                        k0 = kti * 128
                        co = max(0, k0 - q0)
                        sc = scp.tile([128, 2 * QCH], F32, tag="sc")
                        for h in (0, 1):
                            nc.tensor.matmul(
                                sc[:, h * QCH + co:(h + 1) * QCH],
                                kt_sb[h * 64:(h + 1) * 64,
                                          base + k0:base + k0 + 128],
                                qt_sb[h * 64:(h + 1) * 64,
                                          base + q0 + co:base + q0 + QCH],
                                start=True, stop=True,
                            )
                        pt = ptp.tile([128, 2 * QCH], MMDT, tag="pt")
                        sc3 = sc.rearrange("p (h q) -> p h q", h=2)[:, :, co:QCH]
                        pt3 = pt.rearrange("p (h q) -> p h q", h=2)[:, :, co:QCH]
                        nc.scalar.activation(pt3, sc3, EXP)
                        if k0 >= q0:
                            st = pt.rearrange("p (h q) -> p h q", h=2)[
                                :, :, co:co + 128]
                            nc.vector.tensor_tensor(
                                st, st,
                                mask_sb[:, None, :].to_broadcast((128, 2, 128)),
                                MULT)
                        nc.tensor.matmul(
                            att0[:, co:QCH],
                            v_sb[:, vbase + kti, 0:65],
                            pt[:, co:QCH],
                            start=(kti == 0), stop=(kti == n_kt - 1),
                        )
                        nc.tensor.matmul(
                            att1[:, co:QCH],
                            v_sb[:, vbase + kti, 65:130],
                            pt[:, QCH + co:2 * QCH],
                            start=(kti == 0), stop=(kti == n_kt - 1),
                        )
                        if dbg and b == 0 and qc == 0:
                            nc.sync.dma_start(
                                dbg_pt[kti], pt[:].bitcast(F32))
                    # Evacuate PSUM to SBUF immediately (frees the contended
                    # att slots fast), normalize from SBUF, then run this
                    # chunk's output projection as ACT-independent PE filler.
                    cols = slice(base + q0, base + q0 + QCH)
                    au = rcp.tile([65, 2, QCH], F32, tag="au")
                    au0 = au[:, 0, :]
                    au1 = au[:, 1, :]
                    nc.vector.tensor_copy(au0, att0[:])
                    nc.vector.tensor_copy(au1, att1[:])
                    # reciprocal is ~8 cyc/elem/lane: a [1,512] row costs
                    # 4.3us on one lane. Bounce through DRAM to spread the
                    # 2x512 denominators over 128 partitions (recip there is
                    # ~130ns), then broadcast back with a stride-0 DRAM AP.
                    NI = QCH // 128
                    d_dn = drp.tile([2, QCH], F32, tag="ddn")
                    nc.gpsimd.dma_start(d_dn[:, :], au[64:65, :, :])
                    sp = rcp.tile([128, 2, NI], F32, tag="sp")
                    nc.gpsimd.dma_start(
                        sp[:], d_dn.rearrange("h (p i) -> p h i", p=128))
                    rcs = rcp.tile([128, 2, NI], F32, tag="rcs")
                    nc.vector.reciprocal(rcs[:], sp[:])
                    d_rc = drp.tile([2, QCH], F32, tag="drc")
                    nc.gpsimd.dma_start(
                        d_rc.rearrange("h (p i) -> p h i", p=128), rcs[:])
                    bc0 = rcp.tile([64, QCH], F32, tag="bc0")
                    bc1 = rcp.tile([64, QCH], F32, tag="bc1")
                    nc.gpsimd.dma_start(
                        bc0[:], bass.AP(tensor=d_rc.tensor, offset=d_rc.offset,
                                        ap=[[0, 64], [1, QCH]]))
                    nc.gpsimd.dma_start(
                        bc1[:], bass.AP(tensor=d_rc.tensor,
                                        offset=d_rc.offset + QCH,
                                        ap=[[0, 64], [1, QCH]]))
                    if dbg and b == 0 and qc == 0:
                        nc.sync.dma_start(dbg_att[0], au0[:])
                        nc.sync.dma_start(dbg_att[1], au1[:])
                        nc.sync.dma_start(dbg_bc[0], bc0[:])
                        nc.sync.dma_start(dbg_bc[1], bc1[:])
                    nc.vector.tensor_tensor(
                        a_sb[0:64, cols], au0[0:64, :], bc0[:], MULT)
                    a1_t = rcp.tile([64, QCH], MMDT, tag="a1")
                    nc.vector.tensor_tensor(
                        a1_t[:], au1[0:64, :], bc1[:], MULT)
                    nc.sync.dma_start(a_sb[64:128, cols], a1_t[:])
                    # fused output projection for this chunk's token tiles
                    for ti in range(QCH // 128):
                        tt = (base + q0) // 128 + ti
                        o_sb = op.tile([128, D], F32, tag="osb")
                        for oc in range(D // OC):
                            ps = wop.tile([128, OC], F32, tag="wo")
                            nc.tensor.matmul(
                                ps[:],
                                a_sb[:, tt * 128:(tt + 1) * 128],
                                wo_sb[:, oc * OC:(oc + 1) * OC],
                                start=True, stop=True,
                            )
                            dst = o_sb[:, oc * OC:(oc + 1) * OC]
                            if oc % 2 == 0:
                                nc.vector.tensor_copy(dst, ps[:])
                            else:
                                nc.scalar.copy(dst, ps[:])
                        nc.sync.dma_start(
                            out_r[:, tt, :], o_sb[:])

        if dbg:
            nc.sync.dma_start(dbg_qt[:], qt_sb[:].bitcast(F32))
            nc.sync.dma_start(dbg_kt[:], kt_sb[:].bitcast(F32))
            nc.sync.dma_start(dbg_v[:], v_sb[:].bitcast(F32))
            nc.sync.dma_start(dbg_a[:], a_sb[:].bitcast(F32))

    nc.compile()
    return nc


def prep_inputs(in_features, weight_q, weight_k, weight_v, weight_o, cfg: Cfg,
                n_cores=8):
    """Host-side shard/layout prep. Returns per-core input dicts."""
    B, S, D, T, KT = cfg.B, cfg.S, cfg.D, cfg.T, cfg.KT
    if cfg.mm_dt == "bf16":
        import ml_dtypes
        mmnp = ml_dtypes.bfloat16
    else:
        mmnp = np.float32
    x = np.asarray(in_features, dtype=np.float32).reshape(T, D)
    # xT[p, kt, t] = x[t, kt*128 + p]
    xT = np.ascontiguousarray(
        x.T.reshape(KT, 128, T).transpose(1, 0, 2))
    mask = np.triu(np.ones((128, 128), dtype=np.float32))
    wq = np.asarray(weight_q, dtype=np.float32) * (1.0 / np.sqrt(cfg.HD))
    wk = np.asarray(weight_k, dtype=np.float32)
    wv = np.asarray(weight_v, dtype=np.float32)
    wo = np.asarray(weight_o, dtype=np.float32)

    def wslice(w, c):
        # [128, KT, 128]: ws[p, kt, m] = w[c*128 + m, kt*128 + p]
        ws = w[c * 128:(c + 1) * 128, :]                  # [128, D]
        return np.ascontiguousarray(
            ws.T.reshape(KT, 128, 128).transpose(1, 0, 2))

    xT = xT.astype(mmnp)
    in_maps = []
    for c in range(n_cores):
        in_maps.append({
            "xT": xT,
            "wq": wslice(wq, c).astype(mmnp),
            "wk": wslice(wk, c).astype(mmnp),
            "wv": wslice(wv, c).astype(mmnp),
            "wo": np.ascontiguousarray(
                wo[:, c * 128:(c + 1) * 128].T).astype(mmnp),
            "mask": mask,
            "ident": np.eye(128, dtype=mmnp),
        })
    return in_maps


_CACHE = {}


def _get_program(cfg: Cfg):
    key = (cfg.B, cfg.S, cfg.D, cfg.TCH, cfg.QCH, cfg.mm_dt)
    if key not in _CACHE:
        _CACHE[key] = build_program(cfg)
    return _CACHE[key]


def run(inputs, cfg: Cfg, trace=False, trace_kwargs=None):
    import time
    from concourse.bass_utils import run_bass_kernel_spmd
    nc = _get_program(cfg)
    in_maps = prep_inputs(**inputs, cfg=cfg)
    last = None
    for attempt in range(3):
        try:
            res = run_bass_kernel_spmd(
                nc, in_maps, core_ids=list(range(8)), trace=trace,
                **(trace_kwargs or {}))
            break
        except Exception as e:  # transient NRT device wedges happen
            last = e
            time.sleep(10)
    else:
        raise last
    parts = [r["out_p"] for r in res.results]
    out = np.sum(np.stack(parts, 0).astype(np.float64), axis=0)
    return out.astype(np.float32).reshape(cfg.B, cfg.S, cfg.D), res


def kernel(in_features, weight_q, weight_k, weight_v, weight_o):
    cfg = Cfg()
    out, _ = run(dict(in_features=in_features, weight_q=weight_q,
                      weight_k=weight_k, weight_v=weight_v,
                      weight_o=weight_o), cfg)
    return out



# revision 6
# speedup vs baseline: 1.5490x; 1.5490x over previous
"""Causal multi-head self-attention on 8 Trainium2 NeuronCores.

Sharding (2D): core c -> (batch b = c//2, head-group g = c%2 of 8 heads).
Each core computes the full attention for one batch and 8 heads (4
head-pair blocks of 128 qkv dims), then its partial output projection
out_p[b] = A_g @ Wo_g^T; the host sums the 2 partials per batch.
Per-core HBM traffic is ~12 MB (fp16) vs ~67 MB for pure head-parallel.

Per core:
  - x (one batch, [128, KT, S] transposed, fp16) stays resident in SBUF.
  - QT/KT = (x @ W^T)^T per head-pair block in transposed layout
    [128 dims, S tokens] (Wq pre-scaled by 1/sqrt(hd) on host).
  - V computed directly in natural [tokens, dims] layout by making the
    x-tile the stationary operand and streaming all 4 blocks of Wv
    (N=512) -- no PE transposes. A ones-column per head makes each
    attn@V matmul also emit the softmax denominator.
  - scores^T = K Q^T per (head-pair, q-chunk, k-tile); the two heads'
    K=64 matmuls auto-pack into row-tiles (0,0)/(64,0) and run
    concurrently. Causal via per-tile widths + one 128x128 staircase
    mask multiply on diagonal tiles. exp on ScalarE only; all PSUM
    evacuation on VectorE (fp16, 2x mode).
  - softmax denominators spread over 128 partitions via a DRAM bounce
    for the reciprocal, then broadcast back (stride-0 DRAM AP).
  - fused output projection per q-chunk once the last head-pair block's
    attention lands; out written as fp16.
"""

import numpy as np
from contextlib import ExitStack

import concourse.bass as bass
import concourse.mybir as mybir
import concourse.tile as tile
from concourse import bacc

F32 = mybir.dt.float32
F16 = mybir.dt.float16
EXP = mybir.ActivationFunctionType.Exp
MULT = mybir.AluOpType.mult


class Cfg:
    def __init__(self, B=4, S=2048, D=1024, QCH=512, mm_dt="f16"):
        self.B, self.S, self.D = B, S, D
        self.KT = D // 128          # contraction tiles for projections
        self.QCH = QCH              # query chunk for attention
        self.NQC = S // QCH         # q chunks per core
        self.HPB = 4                # head-pair blocks per core (8 heads)
        self.HD = 64
        self.NTT = S // 128         # token tiles per core
        self.mm_dt = mm_dt


def build_program(cfg: Cfg):
    nc = bacc.Bacc("TRN2", target_bir_lowering=False, debug=False)
    S, KT, QCH, NQC, HPB, NTT = (cfg.S, cfg.KT, cfg.QCH, cfg.NQC,
                                 cfg.HPB, cfg.NTT)
    D = cfg.D

    xT_d = nc.dram_tensor("xT", [128, KT, S], F16, kind="ExternalInput")
    wq_d = nc.dram_tensor("wq", [128, KT, HPB, 128], F16, kind="ExternalInput")
    wk_d = nc.dram_tensor("wk", [128, KT, HPB, 128], F16, kind="ExternalInput")
    wv_d = nc.dram_tensor("wv", [128, KT, HPB, 128], F16, kind="ExternalInput")
    wo_d = nc.dram_tensor("wo", [128, HPB, D], F16, kind="ExternalInput")
    mask_d = nc.dram_tensor("mask", [128, 128], F16, kind="ExternalInput")
    out_d = nc.dram_tensor("out_p", [S, D], F16, kind="ExternalOutput")
    out_r = out_d.rearrange("(n p) o -> p n o", p=128)   # [128, NTT, D]

    with tile.TileContext(nc) as tc, ExitStack() as ctx:
        persist = ctx.enter_context(tc.tile_pool(name="persist", bufs=1))

        x_sb = persist.tile([128, KT, S], F16, tag="x")
        qt_sb = persist.tile([128, HPB, S], F16, tag="qt")
        kt_sb = persist.tile([128, HPB, S], F16, tag="kt")
        a_sb = persist.tile([128, HPB, S], F16, tag="a")
        # V natural layout per (token-tile, head-pair block):
        #   cols 0:64 = even head dims, col 64 = 1.0,
        #   cols 65:129 = odd head dims, col 129 = 1.0
        v_sb = persist.tile([128, NTT, HPB, 130], F16, tag="v")
        wq_sb = persist.tile([128, KT, HPB, 128], F16, tag="wq")
        wk_sb = persist.tile([128, KT, HPB, 128], F16, tag="wk")
        wv_sb = persist.tile([128, KT, HPB, 128], F16, tag="wv")
        wo_sb = persist.tile([128, HPB, D], F16, tag="wo")
        mask_sb = persist.tile([128, 128], F16, tag="mask")
        ones128 = persist.tile([128, 1], F16, tag="ones128")
        nc.vector.memset(ones128[:], 1.0)

        for kt2 in range(0, KT, 2):
            nc.sync.dma_start(x_sb[:, kt2:kt2 + 2, :],
                              xT_d[:, kt2:kt2 + 2, :])
        nc.sync.dma_start(wq_sb[:], wq_d[:])
        nc.sync.dma_start(wk_sb[:], wk_d[:])
        nc.sync.dma_start(wv_sb[:], wv_d[:])
        nc.sync.dma_start(wo_sb[:], wo_d[:])
        nc.sync.dma_start(mask_sb[:], mask_d[:])
        nc.vector.tensor_copy(
            v_sb[:, :, :, 64:65],
            ones128[:, None, None, :].to_broadcast((128, NTT, HPB, 1)))
        nc.vector.tensor_copy(
            v_sb[:, :, :, 129:130],
            ones128[:, None, None, :].to_broadcast((128, NTT, HPB, 1)))

        mm = ctx.enter_context(tc.tile_pool(name="mm", bufs=2, space="PSUM"))
        scp = ctx.enter_context(tc.tile_pool(name="scp", bufs=2, space="PSUM"))
        attp = ctx.enter_context(
            tc.tile_pool(name="attp", bufs=2, space="PSUM"))
        ptp = ctx.enter_context(tc.tile_pool(name="ptp", bufs=3))
        rcp = ctx.enter_context(tc.tile_pool(name="rcp", bufs=2))
        op = ctx.enter_context(tc.tile_pool(name="op", bufs=2))
        drp = ctx.enter_context(tc.tile_pool(name="drp", bufs=2, space="DRAM"))

        # ---------------- V projection, natural layout ------------------
        # out[t, (hp, m)] = sum_din x[t, din] * Wv[g*512 + hp*128 + m, din]
        for tt in range(NTT):
            ps = mm.tile([128, 512], F32, tag="mm")
            for kt in range(KT):
                nc.tensor.matmul(
                    ps[:],
                    x_sb[:, kt, tt * 128:(tt + 1) * 128],
                    wv_sb[:, kt, :, :],
                    start=(kt == 0), stop=(kt == KT - 1),
                )
            ps3 = ps.rearrange("p (h m) -> p h m", h=HPB)
            nc.vector.tensor_copy(v_sb[:, tt, :, 0:64], ps3[:, :, 0:64])
            nc.vector.tensor_copy(v_sb[:, tt, :, 65:129], ps3[:, :, 64:128])

        # ------------- per head-pair block: Q/K proj + attention --------
        for hp in range(HPB):
            # Q/K projections for this block, transposed layout
            for w_sb, dst in ((wq_sb, qt_sb), (wk_sb, kt_sb)):
                for tci in range(S // 512):
                    t0 = tci * 512
                    ps = mm.tile([128, 512], F32, tag="mm")
                    for kt in range(KT):
                        nc.tensor.matmul(
                            ps[:],
                            w_sb[:, kt, hp, :],
                            x_sb[:, kt, t0:t0 + 512],
                            start=(kt == 0), stop=(kt == KT - 1),
                        )
                    nc.vector.tensor_copy(dst[:, hp, t0:t0 + 512], ps[:])

            # attention for this block
            for qc in range(NQC):
                q0 = qc * QCH
                n_kt = (q0 + QCH) // 128
                att0 = attp.tile([65, QCH], F32, tag="att")
                att1 = attp.tile([65, QCH], F32, tag="att")
                for kti in range(n_kt):
                    k0 = kti * 128
                    co = max(0, k0 - q0)
                    sc = scp.tile([128, 2, QCH], F32, tag="sc")
                    for h in (0, 1):
                        nc.tensor.matmul(
                            sc[:, h, co:QCH],
                            kt_sb[h * 64:(h + 1) * 64, hp, k0:k0 + 128],
                            qt_sb[h * 64:(h + 1) * 64, hp,
                                  q0 + co:q0 + QCH],
                            start=True, stop=True,
                        )
                    pt = ptp.tile([128, 2, QCH], F16, tag="pt")
                    nc.scalar.activation(pt[:, :, co:QCH], sc[:, :, co:QCH],
                                         EXP)
                    if k0 >= q0:
                        st = pt[:, :, co:co + 128]
                        nc.vector.tensor_tensor(
                            st, st,
                            mask_sb[:, None, :].to_broadcast((128, 2, 128)),
                            MULT)
                    nc.tensor.matmul(
                        att0[:, co:QCH],
                        v_sb[:, kti, hp, 0:65],
                        pt[:, 0, co:QCH],
                        start=(kti == 0), stop=(kti == n_kt - 1),
                    )
                    nc.tensor.matmul(
                        att1[:, co:QCH],
                        v_sb[:, kti, hp, 65:130],
                        pt[:, 1, co:QCH],
                        start=(kti == 0), stop=(kti == n_kt - 1),
                    )
                # normalize: evacuate PSUM fast, reciprocal via DRAM bounce
                cols = slice(q0, q0 + QCH)
                au = rcp.tile([65, 2, QCH], F16, tag="au")
                au0 = au[:, 0, :]
                au1 = au[:, 1, :]
                nc.vector.tensor_copy(au0, att0[:])
                nc.vector.tensor_copy(au1, att1[:])
                NI = QCH // 128
                d_dn = drp.tile([2, QCH], F16, tag="ddn")
                nc.gpsimd.dma_start(d_dn[:, :], au[64:65, :, :])
                sp = rcp.tile([128, 2, NI], F16, tag="sp")
                nc.gpsimd.dma_start(
                    sp[:], d_dn.rearrange("h (p i) -> p h i", p=128))
                rcs = rcp.tile([128, 2, NI], F32, tag="rcs")
                nc.vector.reciprocal(rcs[:], sp[:])
                d_rc = drp.tile([2, QCH], F32, tag="drc")
                nc.gpsimd.dma_start(
                    d_rc.rearrange("h (p i) -> p h i", p=128), rcs[:])
                bc0 = rcp.tile([64, QCH], F32, tag="bc0")
                bc1 = rcp.tile([64, QCH], F32, tag="bc1")
                nc.gpsimd.dma_start(
                    bc0[:], bass.AP(tensor=d_rc.tensor, offset=d_rc.offset,
                                    ap=[[0, 64], [1, QCH]]))
                nc.gpsimd.dma_start(
                    bc1[:], bass.AP(tensor=d_rc.tensor,
                                    offset=d_rc.offset + QCH,
                                    ap=[[0, 64], [1, QCH]]))
                nc.vector.tensor_tensor(
                    a_sb[0:64, hp, cols], au0[0:64, :], bc0[:], MULT)
                a1_t = rcp.tile([64, QCH], F16, tag="a1")
                nc.vector.tensor_tensor(
                    a1_t[:], au1[0:64, :], bc1[:], MULT)
                nc.sync.dma_start(a_sb[64:128, hp, cols], a1_t[:])

                # fused output projection once the last block's attention
                # for this q-chunk is done (a_sb complete for these cols)
                if hp == HPB - 1:
                    for ti in range(QCH // 128):
                        tt = q0 // 128 + ti
                        o_sb = op.tile([128, D], F16, tag="osb")
                        ps0 = mm.tile([128, 512], F32, tag="mm")
                        ps1 = mm.tile([128, 512], F32, tag="mm")
                        for kb in range(HPB):
                            at = a_sb[:, kb, tt * 128:(tt + 1) * 128]
                            nc.tensor.matmul(
                                ps0[:], at, wo_sb[:, kb, 0:512],
                                start=(kb == 0), stop=(kb == HPB - 1))
                            nc.tensor.matmul(
                                ps1[:], at, wo_sb[:, kb, 512:1024],
                                start=(kb == 0), stop=(kb == HPB - 1))
                        nc.vector.tensor_copy(o_sb[:, 0:512], ps0[:])
                        nc.vector.tensor_copy(o_sb[:, 512:1024], ps1[:])
                        nc.sync.dma_start(out_r[:, tt, :], o_sb[:])

    nc.compile()
    return nc


def prep_inputs(in_features, weight_q, weight_k, weight_v, weight_o, cfg: Cfg,
                n_cores=8):
    """Host-side shard/layout prep. Returns per-core input dicts."""
    S, D, KT, HPB = cfg.S, cfg.D, cfg.KT, cfg.HPB
    x = np.asarray(in_features, dtype=np.float32)       # [B, S, D]
    mask = np.triu(np.ones((128, 128), dtype=np.float16))
    wq = np.asarray(weight_q, dtype=np.float32) * (1.0 / np.sqrt(cfg.HD))
    wk = np.asarray(weight_k, dtype=np.float32)
    wv = np.asarray(weight_v, dtype=np.float32)
    wo = np.asarray(weight_o, dtype=np.float32)

    def wblock(w, g):
        # [128, KT, HPB, 128]: ws[p, kt, hp, m] = w[g*512+hp*128+m, kt*128+p]
        blk = w[g * 512:(g + 1) * 512, :]                 # [512, D]
        return np.ascontiguousarray(
            blk.T.reshape(KT, 128, HPB, 128).transpose(1, 0, 2, 3)
        ).astype(np.float16)

    def woblock(g):
        # [128, HPB, D]: wo_sb[p, kb, o] = Wo[o, g*512 + kb*128 + p]
        blk = wo[:, g * 512:(g + 1) * 512]                # [D, 512]
        return np.ascontiguousarray(
            blk.T.reshape(HPB, 128, D).transpose(1, 0, 2)).astype(np.float16)

    xTs = []
    for b in range(cfg.B):
        xb = x[b]                                         # [S, D]
        xTs.append(np.ascontiguousarray(
            xb.T.reshape(KT, 128, S).transpose(1, 0, 2)).astype(np.float16))

    in_maps = []
    for c in range(n_cores):
        b, g = c // 2, c % 2
        in_maps.append({
            "xT": xTs[b],
            "wq": wblock(wq, g),
            "wk": wblock(wk, g),
            "wv": wblock(wv, g),
            "wo": woblock(g),
            "mask": mask,
        })
    return in_maps


_CACHE = {}


def _get_program(cfg: Cfg):
    key = (cfg.B, cfg.S, cfg.D, cfg.QCH, cfg.mm_dt)
    if key not in _CACHE:
        _CACHE[key] = build_program(cfg)
    return _CACHE[key]


def run(inputs, cfg: Cfg, trace=False, trace_kwargs=None):
    import time
    from concourse.bass_utils import run_bass_kernel_spmd
    nc = _get_program(cfg)
    in_maps = prep_inputs(**inputs, cfg=cfg)
    last = None
    for attempt in range(3):
        try:
            res = run_bass_kernel_spmd(
                nc, in_maps, core_ids=list(range(8)), trace=trace,
                **(trace_kwargs or {}))
            break
        except Exception as e:  # transient NRT device wedges happen
            last = e
            time.sleep(10)
    else:
        raise last
    parts = [r["out_p"].astype(np.float32) for r in res.results]
    out = np.stack([parts[2 * b] + parts[2 * b + 1] for b in range(cfg.B)], 0)
    return out.astype(np.float32), res


def kernel(in_features, weight_q, weight_k, weight_v, weight_o):
    cfg = Cfg()
    out, _ = run(dict(in_features=in_features, weight_q=weight_q,
                      weight_k=weight_k, weight_v=weight_v,
                      weight_o=weight_o), cfg)
    return out


# revision 9
# speedup vs baseline: 1.5999x; 1.0329x over previous
"""Causal multi-head self-attention on 8 Trainium2 NeuronCores.

Sharding (2D): core c -> (batch b = c//2, head-group g = c%2 of 8 heads).
Each core computes the full attention for one batch and 8 heads (4
head-pair blocks of 128 qkv dims), then its partial output projection
out_p[b] = A_g @ Wo_g^T; the host sums the 2 partials per batch.
Per-core HBM traffic is ~12 MB (fp16) vs ~67 MB for pure head-parallel.

Per core:
  - x (one batch, [128, KT, S] transposed, fp16) stays resident in SBUF.
  - QT/KT = (x @ W^T)^T per head-pair block in transposed layout
    [128 dims, S tokens] (Wq pre-scaled by 1/sqrt(hd) on host).
  - V computed directly in natural [tokens, dims] layout by making the
    x-tile the stationary operand and streaming all 4 blocks of Wv
    (N=512) -- no PE transposes. A ones-column per head makes each
    attn@V matmul also emit the softmax denominator.
  - scores^T = K Q^T per (head-pair, q-chunk, k-tile); the two heads'
    K=64 matmuls auto-pack into row-tiles (0,0)/(64,0) and run
    concurrently. Causal via per-tile widths + one 128x128 staircase
    mask multiply on diagonal tiles. exp on ScalarE only; all PSUM
    evacuation on VectorE (fp16, 2x mode).
  - softmax denominators spread over 128 partitions via a DRAM bounce
    for the reciprocal, then broadcast back (stride-0 DRAM AP).
  - fused output projection per q-chunk once the last head-pair block's
    attention lands; out written as fp16.
"""

import numpy as np
from contextlib import ExitStack

import concourse.bass as bass
import concourse.mybir as mybir
import concourse.tile as tile
from concourse import bacc

F32 = mybir.dt.float32
F16 = mybir.dt.float16
EXP = mybir.ActivationFunctionType.Exp
MULT = mybir.AluOpType.mult


class Cfg:
    def __init__(self, B=4, S=2048, D=1024, QCH=512, mm_dt="f16"):
        self.B, self.S, self.D = B, S, D
        self.KT = D // 128          # contraction tiles for projections
        self.QCH = QCH              # query chunk for attention
        self.NQC = S // QCH         # q chunks per core
        self.HPB = 4                # head-pair blocks per core (8 heads)
        self.HD = 64
        self.NTT = S // 128         # token tiles per core
        self.mm_dt = mm_dt


def build_program(cfg: Cfg):
    nc = bacc.Bacc("TRN2", target_bir_lowering=False, debug=False)
    S, KT, QCH, NQC, HPB, NTT = (cfg.S, cfg.KT, cfg.QCH, cfg.NQC,
                                 cfg.HPB, cfg.NTT)
    D = cfg.D

    xT_d = nc.dram_tensor("xT", [128, KT, S], F16, kind="ExternalInput")
    wq_d = nc.dram_tensor("wq", [128, KT, HPB, 128], F16, kind="ExternalInput")
    wk_d = nc.dram_tensor("wk", [128, KT, HPB, 128], F16, kind="ExternalInput")
    wv_d = nc.dram_tensor("wv", [128, KT, HPB, 128], F16, kind="ExternalInput")
    wo_d = nc.dram_tensor("wo", [128, HPB, D], F16, kind="ExternalInput")
    mask_d = nc.dram_tensor("mask", [128, 128], F16, kind="ExternalInput")
    out_d = nc.dram_tensor("out_p", [S, D], F16, kind="ExternalOutput")
    out_r = out_d.rearrange("(n p) o -> p n o", p=128)   # [128, NTT, D]

    with tile.TileContext(nc) as tc, ExitStack() as ctx:
        persist = ctx.enter_context(tc.tile_pool(name="persist", bufs=1))

        x_sb = persist.tile([128, KT, S], F16, tag="x")
        qt_sb = persist.tile([128, HPB, S], F16, tag="qt")
        kt_sb = persist.tile([128, HPB, S], F16, tag="kt")
        a_sb = persist.tile([128, HPB, S], F16, tag="a")
        # V natural layout per (token-tile, head-pair block):
        #   cols 0:64 = even head dims, col 64 = 1.0,
        #   cols 65:129 = odd head dims, col 129 = 1.0
        v_sb = persist.tile([128, NTT, HPB, 130], F16, tag="v")
        wq_sb = persist.tile([128, KT, HPB, 128], F16, tag="wq")
        wk_sb = persist.tile([128, KT, HPB, 128], F16, tag="wk")
        wv_sb = persist.tile([128, KT, HPB, 128], F16, tag="wv")
        wo_sb = persist.tile([128, HPB, D], F16, tag="wo")
        mask_sb = persist.tile([128, 128], F16, tag="mask")
        ones128 = persist.tile([128, 1], F16, tag="ones128")
        nc.vector.memset(ones128[:], 1.0)

        nc.sync.dma_start(wv_sb[:], wv_d[:])
        nc.sync.dma_start(wq_sb[:], wq_d[:])
        nc.sync.dma_start(wk_sb[:], wk_d[:])
        nc.sync.dma_start(mask_sb[:], mask_d[:])
        nc.sync.dma_start(wo_sb[:], wo_d[:])
        # split x by token range so the first V-projection matmul can
        # start as soon as the first 256 tokens (+wv) have landed
        for tch in range(0, S, 256):
            nc.sync.dma_start(x_sb[:, :, tch:tch + 256],
                              xT_d[:, :, tch:tch + 256])
        nc.vector.tensor_copy(
            v_sb[:, :, :, 64:65],
            ones128[:, None, None, :].to_broadcast((128, NTT, HPB, 1)))
        nc.vector.tensor_copy(
            v_sb[:, :, :, 129:130],
            ones128[:, None, None, :].to_broadcast((128, NTT, HPB, 1)))

        mm = ctx.enter_context(tc.tile_pool(name="mm", bufs=2, space="PSUM"))
        scp = ctx.enter_context(tc.tile_pool(name="scp", bufs=2, space="PSUM"))
        attp = ctx.enter_context(
            tc.tile_pool(name="attp", bufs=2, space="PSUM"))
        ptp = ctx.enter_context(tc.tile_pool(name="ptp", bufs=3))
        rcp = ctx.enter_context(tc.tile_pool(name="rcp", bufs=2))
        op = ctx.enter_context(tc.tile_pool(name="op", bufs=2))
        drp = ctx.enter_context(tc.tile_pool(name="drp", bufs=2, space="DRAM"))

        # ---------------- V projection, natural layout ------------------
        # out[t, (hp, m)] = sum_din x[t, din] * Wv[g*512 + hp*128 + m, din]
        for tt in range(NTT):
            ps = mm.tile([128, 512], F32, tag="mm")
            for kt in range(KT):
                nc.tensor.matmul(
                    ps[:],
                    x_sb[:, kt, tt * 128:(tt + 1) * 128],
                    wv_sb[:, kt, :, :],
                    start=(kt == 0), stop=(kt == KT - 1),
                )
            ps3 = ps.rearrange("p (h m) -> p h m", h=HPB)
            nc.scalar.copy(v_sb[:, tt, :, 0:64], ps3[:, :, 0:64])
            nc.scalar.copy(v_sb[:, tt, :, 65:129], ps3[:, :, 64:128])

        # ------------- per head-pair block: Q/K proj + attention --------
        for hp in range(HPB):
            # Q/K projections for this block, transposed layout
            for w_sb, dst in ((wq_sb, qt_sb), (wk_sb, kt_sb)):
                for tci in range(S // 512):
                    t0 = tci * 512
                    ps = mm.tile([128, 512], F32, tag="mm")
                    for kt in range(KT):
                        nc.tensor.matmul(
                            ps[:],
                            w_sb[:, kt, hp, :],
                            x_sb[:, kt, t0:t0 + 512],
                            start=(kt == 0), stop=(kt == KT - 1),
                        )
                    if hp == 0:
                        nc.scalar.copy(dst[:, hp, t0:t0 + 512], ps[:])
                    else:
                        nc.vector.tensor_copy(dst[:, hp, t0:t0 + 512], ps[:])

            # attention for this block; for the last block walk q-chunks
            # backwards so the kernel tail ends on the shortest chunk's
            # normalize chain + output projection
            qcs = range(NQC) if hp < HPB - 1 else range(NQC - 1, -1, -1)
            for qc in qcs:
                q0 = qc * QCH
                n_kt = (q0 + QCH) // 128
                att0 = attp.tile([65, QCH], F32, tag="att")
                att1 = attp.tile([65, QCH], F32, tag="att")
                for kti in range(n_kt):
                    k0 = kti * 128
                    co = max(0, k0 - q0)
                    sc = scp.tile([128, 2, QCH], F32, tag="sc")
                    for h in (0, 1):
                        nc.tensor.matmul(
                            sc[:, h, co:QCH],
                            kt_sb[h * 64:(h + 1) * 64, hp, k0:k0 + 128],
                            qt_sb[h * 64:(h + 1) * 64, hp,
                                  q0 + co:q0 + QCH],
                            start=True, stop=True,
                        )
                    pt = ptp.tile([128, 2, QCH], F16, tag="pt")
                    nc.scalar.activation(pt[:, :, co:QCH], sc[:, :, co:QCH],
                                         EXP)
                    if k0 >= q0:
                        st = pt[:, :, co:co + 128]
                        nc.vector.tensor_tensor(
                            st, st,
                            mask_sb[:, None, :].to_broadcast((128, 2, 128)),
                            MULT)
                    nc.tensor.matmul(
                        att0[:, co:QCH],
                        v_sb[:, kti, hp, 0:65],
                        pt[:, 0, co:QCH],
                        start=(kti == 0), stop=(kti == n_kt - 1),
                    )
                    nc.tensor.matmul(
                        att1[:, co:QCH],
                        v_sb[:, kti, hp, 65:130],
                        pt[:, 1, co:QCH],
                        start=(kti == 0), stop=(kti == n_kt - 1),
                    )
                # normalize: evacuate PSUM fast, reciprocal via DRAM bounce
                cols = slice(q0, q0 + QCH)
                au = rcp.tile([65, 2, QCH], F16, tag="au")
                au0 = au[:, 0, :]
                au1 = au[:, 1, :]
                nc.vector.tensor_copy(au0, att0[:])
                nc.vector.tensor_copy(au1, att1[:])
                NI = QCH // 128
                d_dn = drp.tile([2, QCH], F16, tag="ddn")
                nc.gpsimd.dma_start(d_dn[:, :], au[64:65, :, :])
                sp = rcp.tile([128, 2, NI], F16, tag="sp")
                nc.gpsimd.dma_start(
                    sp[:], d_dn.rearrange("h (p i) -> p h i", p=128))
                rcs = rcp.tile([128, 2, NI], F32, tag="rcs")
                nc.vector.reciprocal(rcs[:], sp[:])
                d_rc = drp.tile([2, QCH], F32, tag="drc")
                nc.gpsimd.dma_start(
                    d_rc.rearrange("h (p i) -> p h i", p=128), rcs[:])
                bc0 = rcp.tile([64, QCH], F32, tag="bc0")
                bc1 = rcp.tile([64, QCH], F32, tag="bc1")
                nc.gpsimd.dma_start(
                    bc0[:], bass.AP(tensor=d_rc.tensor, offset=d_rc.offset,
                                    ap=[[0, 64], [1, QCH]]))
                nc.gpsimd.dma_start(
                    bc1[:], bass.AP(tensor=d_rc.tensor,
                                    offset=d_rc.offset + QCH,
                                    ap=[[0, 64], [1, QCH]]))
                nc.vector.tensor_tensor(
                    a_sb[0:64, hp, cols], au0[0:64, :], bc0[:], MULT)
                a1_t = rcp.tile([64, QCH], F16, tag="a1")
                nc.vector.tensor_tensor(
                    a1_t[:], au1[0:64, :], bc1[:], MULT)
                nc.sync.dma_start(a_sb[64:128, hp, cols], a1_t[:])

                # fused output projection once the last block's attention
                # for this q-chunk is done (a_sb complete for these cols)
                if hp == HPB - 1:
                    for ti in range(QCH // 128):
                        tt = q0 // 128 + ti
                        o_sb = op.tile([128, D], F16, tag="osb")
                        ps0 = mm.tile([128, 512], F32, tag="mm")
                        ps1 = mm.tile([128, 512], F32, tag="mm")
                        for kb in range(HPB):
                            at = a_sb[:, kb, tt * 128:(tt + 1) * 128]
                            nc.tensor.matmul(
                                ps0[:], at, wo_sb[:, kb, 0:512],
                                start=(kb == 0), stop=(kb == HPB - 1))
                            nc.tensor.matmul(
                                ps1[:], at, wo_sb[:, kb, 512:1024],
                                start=(kb == 0), stop=(kb == HPB - 1))
                        nc.vector.tensor_copy(o_sb[:, 0:512], ps0[:])
                        nc.vector.tensor_copy(o_sb[:, 512:1024], ps1[:])
                        nc.sync.dma_start(out_r[:, tt, :], o_sb[:])

    nc.compile()
    return nc


def prep_inputs(in_features, weight_q, weight_k, weight_v, weight_o, cfg: Cfg,
                n_cores=8):
    """Host-side shard/layout prep. Returns per-core input dicts."""
    S, D, KT, HPB = cfg.S, cfg.D, cfg.KT, cfg.HPB
    x = np.asarray(in_features, dtype=np.float32)       # [B, S, D]
    mask = np.triu(np.ones((128, 128), dtype=np.float16))
    wq = np.asarray(weight_q, dtype=np.float32) * (1.0 / np.sqrt(cfg.HD))
    wk = np.asarray(weight_k, dtype=np.float32)
    wv = np.asarray(weight_v, dtype=np.float32)
    wo = np.asarray(weight_o, dtype=np.float32)

    def wblock(w, g):
        # [128, KT, HPB, 128]: ws[p, kt, hp, m] = w[g*512+hp*128+m, kt*128+p]
        blk = w[g * 512:(g + 1) * 512, :]                 # [512, D]
        return np.ascontiguousarray(
            blk.T.reshape(KT, 128, HPB, 128).transpose(1, 0, 2, 3)
        ).astype(np.float16)

    def woblock(g):
        # [128, HPB, D]: wo_sb[p, kb, o] = Wo[o, g*512 + kb*128 + p]
        blk = wo[:, g * 512:(g + 1) * 512]                # [D, 512]
        return np.ascontiguousarray(
            blk.T.reshape(HPB, 128, D).transpose(1, 0, 2)).astype(np.float16)

    xTs = []
    for b in range(cfg.B):
        xb = x[b]                                         # [S, D]
        xTs.append(np.ascontiguousarray(
            xb.T.reshape(KT, 128, S).transpose(1, 0, 2)).astype(np.float16))

    in_maps = []
    for c in range(n_cores):
        b, g = c // 2, c % 2
        in_maps.append({
            "xT": xTs[b],
            "wq": wblock(wq, g),
            "wk": wblock(wk, g),
            "wv": wblock(wv, g),
            "wo": woblock(g),
            "mask": mask,
        })
    return in_maps


_CACHE = {}


def _get_program(cfg: Cfg):
    key = (cfg.B, cfg.S, cfg.D, cfg.QCH, cfg.mm_dt)
    if key not in _CACHE:
        _CACHE[key] = build_program(cfg)
    return _CACHE[key]


def run(inputs, cfg: Cfg, trace=False, trace_kwargs=None):
    import time
    from concourse.bass_utils import run_bass_kernel_spmd
    nc = _get_program(cfg)
    in_maps = prep_inputs(**inputs, cfg=cfg)
    last = None
    for attempt in range(3):
        try:
            res = run_bass_kernel_spmd(
                nc, in_maps, core_ids=list(range(8)), trace=trace,
                **(trace_kwargs or {}))
            break
        except Exception as e:  # transient NRT device wedges happen
            last = e
            time.sleep(10)
    else:
        raise last
    parts = [r["out_p"].astype(np.float32) for r in res.results]
    out = np.stack([parts[2 * b] + parts[2 * b + 1] for b in range(cfg.B)], 0)
    return out.astype(np.float32), res


def kernel(in_features, weight_q, weight_k, weight_v, weight_o):
    cfg = Cfg()
    out, _ = run(dict(in_features=in_features, weight_q=weight_q,
                      weight_k=weight_k, weight_v=weight_v,
                      weight_o=weight_o), cfg)
    return out
